# revision 1
# baseline (speedup 1.0000x reference)
"""Trainium2 Bass kernel for nn_CP_LIF (LIF neurons, softplus-parameterized
tau / soft-reset, surrogate-gradient spike forward = hard threshold).

Reference semantics per step (v-space, fp32):
    v   = alpha*v + (1-alpha)*x_t          # alpha = exp(-1/tau), per-neuron
    s   = (v - 1 > 0)                      # forward value of surrogate spike
    v   = v - s*r                          # soft reset, per-neuron r

Device math (w-space): w := (v-1)/r so the threshold is 0 and the reset is 1
for every neuron:
    w_pre = alpha*w_post_prev + bprime*x_t + gamma   (bprime = (1-alpha)/r,
                                                      gamma  = (alpha-1)/r)
    s     = (w_pre > 0)
    w_post= w_pre - s

The serial state is u := alpha*w_post (post-reset, pre-add). A registered
custom DVE micro-op (LIF_RESET_DECAY_ANT: out = ((in0>0) - in0) * in1) fuses
threshold + soft reset + decay-multiply into ONE Vector instruction, so the
recurrence is only 2 DVE instructions per timestep, all on one engine (no
cross-engine serial chain):

Engine split per timestep (all per-core tiles are b-major: 128 batch
partitions x 512 neuron free dim):
    PE  : psum bank = gamma (one K=3 bf16 rank-1 matmul over 3 exact bf16
          pieces) + diag(bprime) @ x_t^T (4 chunk matmuls, fp32) -- off the
          serial path, prefetched several steps ahead
    ACT : evacuate bank PSUM->SBUF (off-path), and
          spikes = Sigmoid(1e30 * W) -> uint8 (exact 0/1, off-path)
    DVE : W = u + xb  ;  u' = ((W>0) - W) * (-alpha)   (the serial path)
    DMA : x^T in (4 steps per 1MB descriptor), spikes out (u8, 4 steps/DMA)

Sharding: neurons split 8 ways (512/core), batch full on every core; the scan
carries no cross-neuron coupling so there is no communication. Measured
~169 us on hardware per core (all 8 run in parallel), bit-exact vs the fp32
CPU reference on the full 100x128x4096 problem.
"""

import sys

import numpy as np

if "/opt/trn_rl_repo" not in sys.path:
    sys.path.insert(0, "/opt/trn_rl_repo")

T, B, N = 100, 128, 4096
NCORES = 8
NLOC = N // NCORES          # 512 neurons per core
NCHUNK = NLOC // 128        # 4 partition-chunks of the neuron dim

DT = 1.0
V_TH = 1.0
TAU_MIN = 1e-3
R_MIN = 1e-6

_NC_CACHE = {}


KB = 4  # timesteps batched per DMA (in and out)
GPZ = 0  # GPSIMD add-slice disabled: measured no gain (DVE-GPSIMD SBUF port contention)

_LIF_OP = None


def _register_lif_op():
    """Custom DVE op: out = ((in0 > 0) - in0) * in1.

    With in0 = W (pre-reset membrane, w-space) and in1 = -alpha, this computes
    alpha*(W - spike) = the decayed post-reset state, fusing threshold, reset
    and decay-multiply into one Vector instruction (3 ALU stages).
    """
    global _LIF_OP
    if _LIF_OP is not None:
        return _LIF_OP
    import concourse.dve_ops as dve_ops
    from concourse.dve_ops import DveOp, OPS, CUSTOM_DVE_SPECS, _SUB_OPCODE_FOR_NAME
    from concourse.dve_spec import Spec, Src0, Src1, Zero, lower
    from concourse.dve_uop import DveOpSpec

    name = "LIF_RESET_DECAY_ANT"
    if name in _SUB_OPCODE_FOR_NAME:
        _LIF_OP = next(op for op in OPS if op.name == name)
        return _LIF_OP

    spec = Spec(
        body=((Src0 > Zero) - Src0) * Src1,
        reference=lambda in0, in1, c0, c1, c2: (
            ((in0 > 0).astype(np.float32) - in0) * in1
        ).astype(np.float32),
    )
    row = dve_ops._CUSTOM_DVE_ROW_BASE + len(OPS)
    assert row < 0x20
    shas = {}
    for ver in ("v3", "v4"):
        tmp = DveOpSpec(name=name, opcode=row, uops=lower(spec, ver=ver), rd1_en=True)
        shas[ver] = tmp.sha(ver)
    op = DveOp(name, spec, subdim=False, uops_sha=shas)
    OPS.append(op)
    CUSTOM_DVE_SPECS[name] = spec
    _SUB_OPCODE_FOR_NAME[name] = row
    _LIF_OP = op
    return op


def _build_nc(n_steps=T):
    import concourse.bacc as bacc
    import concourse.tile as tile
    from concourse import mybir

    f32 = mybir.dt.float32
    bf16 = mybir.dt.bfloat16
    u8 = mybir.dt.uint8
    Op = mybir.AluOpType

    assert n_steps % KB == 0

    lif_op = _register_lif_op()

    nc = bacc.Bacc("TRN2", target_bir_lowering=False, debug=False)

    xT = nc.dram_tensor("xT", [n_steps, NLOC, B], f32, kind="ExternalInput").ap()
    negalpha = nc.dram_tensor("negalpha", [B, NLOC], f32, kind="ExternalInput").ap()
    gamma3 = nc.dram_tensor("gamma3", [3, NLOC], bf16, kind="ExternalInput").ap()
    ones = nc.dram_tensor("ones", [3, B], bf16, kind="ExternalInput").ap()
    diagb = nc.dram_tensor("diagb", [NCHUNK, 128, 128], f32, kind="ExternalInput").ap()
    uinit = nc.dram_tensor("uinit", [B, NLOC], f32, kind="ExternalInput").ap()
    sout = nc.dram_tensor("sout", [n_steps, B, NLOC], u8, kind="ExternalOutput").ap()

    _emit(nc, tile, mybir, lif_op, xT, negalpha, gamma3, ones, diagb, uinit,
          sout, n_steps, reps=1)

    nc.compile()
    return nc


def _emit(nc, tile, mybir, lif_op, xT, negalpha, gamma3, ones, diagb, uinit,
          sout, n_steps, reps=1):
    f32 = mybir.dt.float32
    bf16 = mybir.dt.bfloat16
    u8 = mybir.dt.uint8
    Op = mybir.AluOpType
    from contextlib import nullcontext

    with tile.TileContext(nc) as tc:
        with (
            tc.tile_pool(name="const", bufs=1) as const,
            tc.tile_pool(name="xp", bufs=8) as xpool,
            tc.tile_pool(name="up", bufs=4) as upool,
            tc.tile_pool(name="wp", bufs=4) as wpool,
            tc.tile_pool(name="xb", bufs=4) as xbpool,
            tc.tile_pool(name="sp", bufs=4) as spool,
            tc.tile_pool(name="bank", bufs=8, space="PSUM") as bankpool,
        ):
            na_t = const.tile([B, NLOC], f32)
            nc.sync.dma_start(na_t[:], negalpha)
            g_t = const.tile([3, NLOC], bf16)
            nc.sync.dma_start(g_t[:], gamma3)
            on_t = const.tile([3, B], bf16)
            nc.sync.dma_start(on_t[:], ones)
            db_t = const.tile([128, NCHUNK * 128], f32)
            for c in range(NCHUNK):
                nc.sync.dma_start(db_t[:, c * 128:(c + 1) * 128], diagb[c])

            rep_cm = tc.For_i(0, reps, 1) if reps > 1 else nullcontext()
            with rep_cm:
                u_t = upool.tile([B, NLOC], f32)
                nc.sync.dma_start(u_t[:], uinit)
                body(tc, nc, lif_op, mybir, xT, sout, n_steps,
                     xpool, upool, wpool, xbpool, spool, bankpool,
                     na_t, g_t, on_t, db_t, u_t)


def body(tc, nc, lif_op, mybir, xT, sout, n_steps,
         xpool, upool, wpool, xbpool, spool, bankpool,
         na_t, g_t, on_t, db_t, u_t):
    f32 = mybir.dt.float32
    u8 = mybir.dt.uint8
    Op = mybir.AluOpType
    if True:
            for t0 in range(0, n_steps, KB):
                # one DMA: KB steps of x^T -> (128, KB*NCHUNK*128) SBUF tile
                xt = xpool.tile([128, KB * NLOC], f32)
                src = xT[t0:t0 + KB].rearrange("u (c p) b -> p u c b", p=128)
                dst = xt[:].rearrange("p (u c b) -> p u c b", u=KB, c=NCHUNK)
                nc.sync.dma_start(dst, src)

                # one output tile for KB steps of spikes
                s_t = spool.tile([B, KB * NLOC], u8)

                for k in range(KB):
                    t = t0 + k
                    # --- PE: bank = gamma + diag(bprime) @ x_t (b-major) ---
                    bk = bankpool.tile([B, NLOC], f32)
                    nc.tensor.matmul(bk[:], on_t[:], g_t[:], start=True, stop=False)
                    for c in range(NCHUNK):
                        nc.tensor.matmul(
                            bk[:, c * 128:(c + 1) * 128],
                            xt[:, (k * NCHUNK + c) * 128:(k * NCHUNK + c + 1) * 128],
                            db_t[:, c * 128:(c + 1) * 128],
                            start=False,
                            stop=(c == NCHUNK - 1),
                        )

                    # --- ACT: evacuate bank to SBUF (off the serial path) ---
                    xb_t = xbpool.tile([B, NLOC], f32)
                    nc.scalar.copy(xb_t[:], bk[:])

                    # --- serial path: W = u + xb (DVE + a GPSIMD column
                    # slice so the Vector engine's add shrinks) ---
                    w_t = wpool.tile([B, NLOC], f32)
                    if GPZ:
                        nc.vector.tensor_tensor(
                            w_t[:, :NLOC - GPZ], u_t[:, :NLOC - GPZ],
                            xb_t[:, :NLOC - GPZ], Op.add)
                        nc.gpsimd.tensor_tensor(
                            w_t[:, NLOC - GPZ:], u_t[:, NLOC - GPZ:],
                            xb_t[:, NLOC - GPZ:], Op.add)
                    else:
                        nc.vector.tensor_tensor(w_t[:], u_t[:], xb_t[:], Op.add)

                    # --- ACT: spikes (exact 0/1 after u8 round) ---
                    nc.scalar.activation(
                        s_t[:, k * NLOC:(k + 1) * NLOC], w_t[:],
                        mybir.ActivationFunctionType.Sigmoid,
                        bias=0.0, scale=1e30,
                    )

                    # --- DVE: fused threshold+reset+decay ---
                    u_t = upool.tile([B, NLOC], f32)
                    nc.vector._custom_dve(
                        lif_op, out=u_t[:], in0=w_t[:], in1=na_t[:]
                    )

                # one DMA: KB steps of spikes out
                nc.sync.dma_start(
                    sout[t0:t0 + KB].rearrange("u p n -> p u n"),
                    s_t[:].rearrange("p (u n) -> p u n", u=KB),
                )


def _get_nc(n_steps=T):
    if n_steps not in _NC_CACHE:
        _NC_CACHE[n_steps] = _build_nc(n_steps)
    return _NC_CACHE[n_steps]


def _derive_params(tau_raw, r_raw):
    """Per-neuron constants, fp32, matching the jax reference on CPU."""
    tr = np.asarray(tau_raw, dtype=np.float32)
    rr = np.asarray(r_raw, dtype=np.float32)
    # softplus(x) = logaddexp(x, 0); matches jax CPU to <=1 ulp
    tau = np.logaddexp(np.float32(0.0), tr).astype(np.float32) + np.float32(TAU_MIN)
    alpha = np.exp(-np.float32(DT) / tau).astype(np.float32)
    r = np.logaddexp(np.float32(0.0), rr).astype(np.float32) + np.float32(R_MIN)
    beta = np.float32(1.0) - alpha
    bprime = beta / r
    gamma = -bprime                     # (alpha-1)/r == -(1-alpha)/r exactly
    minit = (np.float32(0.0) - np.float32(V_TH)) / r   # w(v=0) = -1/r
    return alpha, r, bprime, gamma, minit


def _core_inputs(x, alpha, bprime, gamma, minit, core, n_steps):
    sl = slice(core * NLOC, (core + 1) * NLOC)
    xT = np.ascontiguousarray(
        x[:n_steps, :, sl].transpose(0, 2, 1), dtype=np.float32
    )
    import ml_dtypes

    na = np.ascontiguousarray(
        np.broadcast_to(-alpha[sl], (B, NLOC)), dtype=np.float32
    )
    gl = gamma[sl].astype(np.float32)
    g1 = gl.astype(ml_dtypes.bfloat16)
    g2 = (gl - g1.astype(np.float32)).astype(ml_dtypes.bfloat16)
    g3 = (gl - g1.astype(np.float32) - g2.astype(np.float32)).astype(
        ml_dtypes.bfloat16
    )
    g = np.stack([g1, g2, g3]).astype(ml_dtypes.bfloat16)
    on = np.ones((3, B), dtype=ml_dtypes.bfloat16)
    db = np.zeros((NCHUNK, 128, 128), dtype=np.float32)
    bp = bprime[sl]
    for c in range(NCHUNK):
        np.fill_diagonal(db[c], bp[c * 128:(c + 1) * 128])
    u0 = (alpha[sl] * minit[sl]).astype(np.float32)  # alpha * w_init
    mi = np.ascontiguousarray(np.broadcast_to(u0, (B, NLOC)), dtype=np.float32)
    return {
        "xT": xT,
        "negalpha": na,
        "gamma3": g,
        "ones": on,
        "diagb": db,
        "uinit": mi,
    }


def _run(x, tau_raw, r_raw, n_steps=T, trace=False, **run_kwargs):
    from concourse.bass_utils import run_bass_kernel_spmd

    alpha, r, bprime, gamma, minit = _derive_params(tau_raw, r_raw)
    in_maps = [
        _core_inputs(x, alpha, bprime, gamma, minit, c, n_steps)
        for c in range(NCORES)
    ]
    nc = _get_nc(n_steps)
    res = run_bass_kernel_spmd(
        nc, in_maps, core_ids=list(range(NCORES)), trace=trace, **run_kwargs
    )
    shards = [res.results[c]["sout"] for c in range(NCORES)]
    out = np.concatenate(shards, axis=-1).astype(np.float32)
    return out, res


def kernel(x, tau_raw, r_raw):
    x = np.asarray(x, dtype=np.float32)
    tau_raw = np.asarray(tau_raw, dtype=np.float32)
    r_raw = np.asarray(r_raw, dtype=np.float32)
    last = None
    for attempt in range(3):
        try:
            out, _ = _run(x, tau_raw, r_raw)
            return out
        except Exception as e:  # transient NRT device errors observed rarely
            last = e
            import time as _time

            _time.sleep(2.0 * (attempt + 1))
    raise last



# revision 4
# speedup vs baseline: 1.4593x; 1.4593x over previous
"""Trainium2 Bass kernel for nn_CP_LIF (LIF neurons, softplus-parameterized
tau / soft-reset, surrogate-gradient spike forward = hard threshold).

Reference semantics per step (v-space, fp32):
    v   = alpha*v + (1-alpha)*x_t          # alpha = exp(-1/tau), per-neuron
    s   = (v - 1 > 0)                      # forward value of surrogate spike
    v   = v - s*r                          # soft reset, per-neuron r

Device math (P-space): Z := (v' - 1)/(1-alpha) + 1, so the threshold is the
constant 1 and the input current is RAW x (no per-neuron input scaling):
    Z_{t+1} = ((Z_t - 1) - (Z_t > 1)*C1) * C0 + x_{t+1}
    s_t     = (Z_t > 1)
with per-neuron constants C0 = alpha, C1 = 1/bprime (bprime = (1-alpha)/r).

The key trick: a single custom DVE instruction evaluates KB timesteps of the
recurrence via in-instruction self-feedback. The Z buffer holds, per neuron
chunk, KB+1 blocks of 128 batch columns: [Z_t0 | x_{t0+1} .. x_{t0+KB}].
The op's in0 AP covers blocks 0..KB-1 while its out (and in1) AP covers
blocks 1..KB — the DVE streams the free dimension in order at 1 elem/cycle,
so the output block written ~120 cycles earlier is re-read as the next
step's state (verified bit-exact on HW). x is overwritten by Z in place.

Engines per group of KB steps (n-major layout, 4 chunks of 128 neurons):
    DVE : 4 chunk ops, FD = KB*128 each   (the entire recurrence)
    ACT : 4 boundary copies (Z_tKB -> next tile block 0, hidden under DVE)
          + spikes = Sigmoid(1e30*Z - 1e30) -> uint8 over all 4 chunks
    PE  : completely idle
    DMA : x in (one contiguous host-prearranged slab per group),
          spikes out (contiguous u8 slab per group)

Sharding: neurons split 8 ways (512/core), batch full on every core; no
cross-core communication.
"""

import sys

import numpy as np

if "/opt/trn_rl_repo" not in sys.path:
    sys.path.insert(0, "/opt/trn_rl_repo")

T, B, N = 100, 128, 4096
NCORES = 8
NLOC = N // NCORES          # 512 neurons per core
NCH = NLOC // 128           # 4 partition-chunks of the neuron dim
BLK = 128                   # batch block width (one timestep column block)

DT = 1.0
V_TH = 1.0
TAU_MIN = 1e-3
R_MIN = 1e-6

KB = 10                     # timesteps per DVE instruction / per group

_NC_CACHE = {}
_LIF_OP = None


def _register_lif_op():
    """Custom DVE op: out = ((in0 - 1) - (in0 > 1)*C1)*C0 + in1.

    With in0 = Z_t (prev state block), in1 = x_{t+1}, C0 = alpha (per
    partition), C1 = 1/bprime (per partition) this computes Z_{t+1}; the
    in0/out APs overlap shifted by one 128-col block so one instruction
    evaluates KB serial timesteps.
    """
    global _LIF_OP
    if _LIF_OP is not None:
        return _LIF_OP
    import concourse.dve_ops as dve_ops
    from concourse.dve_ops import DveOp, OPS, CUSTOM_DVE_SPECS, _SUB_OPCODE_FOR_NAME
    from concourse.dve_spec import Spec, Src0, Src1, C0, C1, One, lower
    from concourse.dve_uop import DveOpSpec

    name = "LIF_STREAM_ANT"
    if name in _SUB_OPCODE_FOR_NAME:
        _LIF_OP = next(op for op in OPS if op.name == name)
        return _LIF_OP

    z = Src0
    s = z > One
    spec = Spec(
        body=((z - One) - s * C1) * C0 + Src1,
        reference=lambda in0, in1, s0, s1, imm2: (
            ((in0 - 1.0) - (in0 > 1.0).astype(np.float32) * s1) * s0 + in1
        ).astype(np.float32),
    )
    row = dve_ops._CUSTOM_DVE_ROW_BASE + len(OPS)
    assert row < 0x20
    shas = {}
    for ver in ("v3", "v4"):
        tmp = DveOpSpec(name=name, opcode=row, uops=lower(spec, ver=ver),
                        rd1_en=True)
        shas[ver] = tmp.sha(ver)
    op = DveOp(name, spec, subdim=False, uops_sha=shas)
    OPS.append(op)
    CUSTOM_DVE_SPECS[name] = spec
    _SUB_OPCODE_FOR_NAME[name] = row
    _LIF_OP = op
    return op


def _build_nc(n_steps=T):
    import concourse.bacc as bacc
    import concourse.tile as tile
    from concourse import mybir

    f32 = mybir.dt.float32
    u8 = mybir.dt.uint8

    assert n_steps % KB == 0
    G = n_steps // KB

    lif_op = _register_lif_op()

    nc = bacc.Bacc("TRN2", target_bir_lowering=False, debug=False)

    xg = nc.dram_tensor("xg", [G, 128, NCH * KB * BLK], f32,
                        kind="ExternalInput").ap()
    z0 = nc.dram_tensor("z0", [128, NCH * BLK], f32, kind="ExternalInput").ap()
    alpha4 = nc.dram_tensor("alpha4", [128, NCH], f32, kind="ExternalInput").ap()
    rinv4 = nc.dram_tensor("rinv4", [128, NCH], f32, kind="ExternalInput").ap()
    sg = nc.dram_tensor("sg", [G, 128, NCH * KB * BLK], u8,
                        kind="ExternalOutput").ap()

    _emit(nc, tile, mybir, lif_op, xg, z0, alpha4, rinv4, sg, n_steps, reps=1)

    nc.compile()
    return nc


def _emit(nc, tile, mybir, lif_op, xg, z0, alpha4, rinv4, sg, n_steps, reps=1):
    f32 = mybir.dt.float32
    u8 = mybir.dt.uint8
    from contextlib import nullcontext

    G = n_steps // KB
    SEC = (KB + 1) * BLK          # columns per chunk section in the Z tile

    with tile.TileContext(nc) as tc:
        with (
            tc.tile_pool(name="const", bufs=1) as const,
            tc.tile_pool(name="zp", bufs=3) as zpool,
            tc.tile_pool(name="sp", bufs=3) as spool,
        ):
            a_t = const.tile([128, NCH], f32)
            nc.sync.dma_start(a_t[:], alpha4)
            ri_t = const.tile([128, NCH], f32)
            nc.sync.dma_start(ri_t[:], rinv4)
            nb_t = const.tile([128, 1], f32)
            nc.gpsimd.memset(nb_t[:], -1.0e30)

            rep_cm = tc.For_i(0, reps, 1) if reps > 1 else nullcontext()
            with rep_cm:
                zprev = None
                for g in range(G):
                    zt = zpool.tile([128, NCH * SEC], f32)

                    # state into block 0 of each chunk section
                    if zprev is None:
                        dst = zt[:].rearrange("p (c y) -> p c y", c=NCH)[:, :, :BLK]
                        src = z0.rearrange("p (c b) -> p c b", c=NCH)
                        nc.sync.dma_start(dst, src)
                    else:
                        for c in range(NCH):
                            nc.scalar.copy(
                                zt[:, c * SEC:c * SEC + BLK],
                                zprev[:, c * SEC + KB * BLK:(c + 1) * SEC],
                            )

                    # x for this group into blocks 1..KB (contiguous src slab)
                    dst = zt[:].rearrange("p (c y) -> p c y", c=NCH)[:, :, BLK:]
                    src = xg[g].rearrange("p (c y) -> p c y", c=NCH)
                    nc.sync.dma_start(dst, src)

                    # the recurrence: one DVE op per chunk, KB steps each
                    for c in range(NCH):
                        base = c * SEC
                        nc.vector._custom_dve(
                            lif_op,
                            out=zt[:, base + BLK:base + SEC],
                            in0=zt[:, base:base + KB * BLK],
                            in1=zt[:, base + BLK:base + SEC],
                            s0=a_t[:, c:c + 1],
                            s1=ri_t[:, c:c + 1],
                        )

                    # spikes: s = (Z > 1) as exact 0/1 u8
                    st = spool.tile([128, NCH * KB * BLK], u8)
                    nc.scalar.activation(
                        st[:].rearrange("p (c y) -> p c y", c=NCH),
                        zt[:].rearrange("p (c y) -> p c y", c=NCH)[:, :, BLK:],
                        mybir.ActivationFunctionType.Sigmoid,
                        bias=nb_t[:, 0:1],
                        scale=1.0e30,
                    )
                    nc.sync.dma_start(sg[g], st[:])

                    zprev = zt


def _get_nc(n_steps=T):
    if n_steps not in _NC_CACHE:
        _NC_CACHE[n_steps] = _build_nc(n_steps)
    return _NC_CACHE[n_steps]


def _derive_params(tau_raw, r_raw):
    """Per-neuron constants, fp32 softplus path matching jax CPU exactly."""
    tr = np.asarray(tau_raw, dtype=np.float32)
    rr = np.asarray(r_raw, dtype=np.float32)
    tau = np.logaddexp(np.float32(0.0), tr).astype(np.float32) + np.float32(TAU_MIN)
    alpha = np.exp(-np.float32(DT) / tau).astype(np.float32)
    r = np.logaddexp(np.float32(0.0), rr).astype(np.float32) + np.float32(R_MIN)
    beta = np.float32(1.0) - alpha
    bprime = beta / r
    # C1 = 1/bprime = r/beta; z0 = (1 - 1/beta)/alpha + 1 in f64 then f32
    c1 = (r.astype(np.float64) / beta.astype(np.float64)).astype(np.float32)
    z0 = (
        (1.0 - 1.0 / beta.astype(np.float64)) / alpha.astype(np.float64) + 1.0
    ).astype(np.float32)
    return alpha, c1, z0


def _core_inputs(x, alpha, c1, z0, core, n_steps):
    G = n_steps // KB
    sl = slice(core * NLOC, (core + 1) * NLOC)
    # x[:, :, sl] is [T, B, 512] -> [G, KB(t), B(b), NCH(c), 128(p)]
    #   -> device slab [G, 128(p), NCH(c), KB(t), B(b)]
    xs = x[:n_steps, :, sl].reshape(G, KB, B, NCH, 128)
    xg = np.ascontiguousarray(xs.transpose(0, 4, 3, 1, 2), dtype=np.float32)
    xg = xg.reshape(G, 128, NCH * KB * BLK)

    al = alpha[sl].reshape(NCH, 128)     # [c, p]
    a4 = np.ascontiguousarray(al.T, dtype=np.float32)          # [p, c]
    r4 = np.ascontiguousarray(c1[sl].reshape(NCH, 128).T, dtype=np.float32)
    # z0 block: [p, (c, b)] broadcast along b
    z0l = z0[sl].reshape(NCH, 128).T      # [p, c]
    z0b = np.ascontiguousarray(
        np.broadcast_to(z0l[:, :, None], (128, NCH, BLK)), dtype=np.float32
    ).reshape(128, NCH * BLK)
    return {"xg": xg, "z0": z0b, "alpha4": a4, "rinv4": r4}


def _run(x, tau_raw, r_raw, n_steps=T, trace=False, **run_kwargs):
    from concourse.bass_utils import run_bass_kernel_spmd

    alpha, c1, z0 = _derive_params(tau_raw, r_raw)
    in_maps = [
        _core_inputs(x, alpha, c1, z0, c, n_steps) for c in range(NCORES)
    ]
    nc = _get_nc(n_steps)
    res = run_bass_kernel_spmd(
        nc, in_maps, core_ids=list(range(NCORES)), trace=trace, **run_kwargs
    )
    G = n_steps // KB
    shards = []
    for c in range(NCORES):
        sgc = res.results[c]["sg"].reshape(G, 128, NCH, KB, BLK)
        # [g, p, c, t, b] -> [t_abs, b, n_local = c*128 + p]
        sc = sgc.transpose(0, 3, 4, 2, 1).reshape(n_steps, B, NLOC)
        shards.append(sc)
    out = np.concatenate(shards, axis=-1).astype(np.float32)
    return out, res


def kernel(x, tau_raw, r_raw):
    x = np.asarray(x, dtype=np.float32)
    tau_raw = np.asarray(tau_raw, dtype=np.float32)
    r_raw = np.asarray(r_raw, dtype=np.float32)
    last = None
    for attempt in range(3):
        try:
            out, _ = _run(x, tau_raw, r_raw)
            return out
        except Exception as e:  # transient NRT device errors observed rarely
            last = e
            import time as _time

            _time.sleep(2.0 * (attempt + 1))
    raise last


# revision 5
# speedup vs baseline: 1.5470x; 1.0601x over previous
"""Trainium2 Bass kernel for nn_CP_LIF (LIF neurons, softplus-parameterized
tau / soft-reset, surrogate-gradient spike forward = hard threshold).

Reference semantics per step (v-space, fp32):
    v   = alpha*v + (1-alpha)*x_t          # alpha = exp(-1/tau), per-neuron
    s   = (v - 1 > 0)                      # forward value of surrogate spike
    v   = v - s*r                          # soft reset, per-neuron r

Device math (P-space): Z := (v' - 1)/(1-alpha) + 1, so the threshold is the
constant 1 and the input current is RAW x (no per-neuron input scaling):
    Z_{t+1} = ((Z_t - 1) - (Z_t > 1)*C1) * C0 + x_{t+1}
    s_t     = (Z_t > 1)
with per-neuron constants C0 = alpha, C1 = 1/bprime (bprime = (1-alpha)/r).

The key trick: a single custom DVE instruction evaluates KB timesteps of the
recurrence via in-instruction self-feedback. The Z buffer holds, per neuron
chunk, KB+1 blocks of 128 batch columns: [Z_t0 | x_{t0+1} .. x_{t0+KB}].
The op's in0 AP covers blocks 0..KB-1 while its out (and in1) AP covers
blocks 1..KB — the DVE streams the free dimension in order at 1 elem/cycle,
so the output block written ~120 cycles earlier is re-read as the next
step's state (verified bit-exact on HW). x is overwritten by Z in place.

Engines per group of KB steps (n-major layout, 4 chunks of 128 neurons):
    DVE : 4 chunk ops, FD = KB*128 each   (the entire recurrence)
    ACT : 4 boundary copies (Z_tKB -> next tile block 0, hidden under DVE)
          + spikes = Sigmoid(1e30*Z - 1e30) -> uint8 over all 4 chunks
    PE  : completely idle
    DMA : x in (one contiguous host-prearranged slab per group),
          spikes out (contiguous u8 slab per group)

Sharding: neurons split 8 ways (512/core), batch full on every core; no
cross-core communication.
"""

import sys

import numpy as np

if "/opt/trn_rl_repo" not in sys.path:
    sys.path.insert(0, "/opt/trn_rl_repo")

T, B, N = 100, 128, 4096
NCORES = 8
NLOC = N // NCORES          # 512 neurons per core
NCH = NLOC // 128           # 4 partition-chunks of the neuron dim
BLK = 128                   # batch block width (one timestep column block)

DT = 1.0
V_TH = 1.0
TAU_MIN = 1e-3
R_MIN = 1e-6

KB = 10                     # timesteps per DVE instruction / per group

_NC_CACHE = {}
_LIF_OP = None


def _register_lif_op():
    """Custom DVE op: out = ((in0 - 1) - (in0 > 1)*C1)*C0 + in1.

    With in0 = Z_t (prev state block), in1 = x_{t+1}, C0 = alpha (per
    partition), C1 = 1/bprime (per partition) this computes Z_{t+1}; the
    in0/out APs overlap shifted by one 128-col block so one instruction
    evaluates KB serial timesteps.
    """
    global _LIF_OP
    if _LIF_OP is not None:
        return _LIF_OP
    import concourse.dve_ops as dve_ops
    from concourse.dve_ops import DveOp, OPS, CUSTOM_DVE_SPECS, _SUB_OPCODE_FOR_NAME
    from concourse.dve_spec import Spec, Src0, Src1, C0, C1, One, lower
    from concourse.dve_uop import DveOpSpec

    name = "LIF_STREAM_ANT"
    if name in _SUB_OPCODE_FOR_NAME:
        _LIF_OP = next(op for op in OPS if op.name == name)
        return _LIF_OP

    z = Src0
    s = z > One
    spec = Spec(
        body=((z - One) - s * C1) * C0 + Src1,
        reference=lambda in0, in1, s0, s1, imm2: (
            ((in0 - 1.0) - (in0 > 1.0).astype(np.float32) * s1) * s0 + in1
        ).astype(np.float32),
    )
    row = dve_ops._CUSTOM_DVE_ROW_BASE + len(OPS)
    assert row < 0x20
    shas = {}
    for ver in ("v3", "v4"):
        tmp = DveOpSpec(name=name, opcode=row, uops=lower(spec, ver=ver),
                        rd1_en=True)
        shas[ver] = tmp.sha(ver)
    op = DveOp(name, spec, subdim=False, uops_sha=shas)
    OPS.append(op)
    CUSTOM_DVE_SPECS[name] = spec
    _SUB_OPCODE_FOR_NAME[name] = row
    _LIF_OP = op
    return op


def _build_nc(n_steps=T):
    import concourse.bacc as bacc
    import concourse.tile as tile
    from concourse import mybir

    f32 = mybir.dt.float32
    u8 = mybir.dt.uint8

    assert n_steps % KB == 0
    G = n_steps // KB

    lif_op = _register_lif_op()

    nc = bacc.Bacc("TRN2", target_bir_lowering=False, debug=False)

    xg = nc.dram_tensor("xg", [G, 128, NCH * KB * BLK], f32,
                        kind="ExternalInput").ap()
    z0 = nc.dram_tensor("z0", [128, NCH * BLK], f32, kind="ExternalInput").ap()
    alpha4 = nc.dram_tensor("alpha4", [128, NCH], f32, kind="ExternalInput").ap()
    rinv4 = nc.dram_tensor("rinv4", [128, NCH], f32, kind="ExternalInput").ap()
    sg = nc.dram_tensor("sg", [G, 128, NCH * KB * BLK], u8,
                        kind="ExternalOutput").ap()

    _emit(nc, tile, mybir, lif_op, xg, z0, alpha4, rinv4, sg, n_steps, reps=1)

    nc.compile()
    return nc


def _emit(nc, tile, mybir, lif_op, xg, z0, alpha4, rinv4, sg, n_steps, reps=1):
    f32 = mybir.dt.float32
    u8 = mybir.dt.uint8
    from contextlib import nullcontext

    G = n_steps // KB
    SEC = (KB + 1) * BLK          # columns per chunk section in the Z tile
    PF = 2                        # x-DMA prefetch depth (groups ahead)

    with tile.TileContext(nc) as tc:
        with (
            tc.tile_pool(name="const", bufs=1) as const,
            tc.tile_pool(name="zp", bufs=PF + 3) as zpool,
            tc.tile_pool(name="sp", bufs=3) as spool,
        ):
            a_t = const.tile([128, NCH], f32)
            nc.sync.dma_start(a_t[:], alpha4)
            ri_t = const.tile([128, NCH], f32)
            nc.sync.dma_start(ri_t[:], rinv4)
            nb_t = const.tile([128, 1], f32)
            nc.gpsimd.memset(nb_t[:], -1.0e30)

            rep_cm = tc.For_i(0, reps, 1) if reps > 1 else nullcontext()
            with rep_cm:
                zts = {}

                def alloc_and_fetch(i):
                    zt = zpool.tile([128, NCH * SEC], f32)
                    zts[i] = zt
                    dst = zt[:].rearrange("p (c y) -> p c y", c=NCH)[:, :, BLK:]
                    src = xg[i].rearrange("p (c y) -> p c y", c=NCH)
                    nc.sync.dma_start(dst, src)

                # prologue: prefetch x for the first PF+1 groups, seed state
                for i in range(min(PF + 1, G)):
                    alloc_and_fetch(i)
                dst = zts[0][:].rearrange("p (c y) -> p c y", c=NCH)[:, :, :BLK]
                nc.sync.dma_start(dst, z0.rearrange("p (c b) -> p c b", c=NCH))

                for g in range(G):
                    zt = zts[g]

                    # the recurrence: one DVE op per chunk, KB steps each,
                    # in-instruction self-feedback across the KB blocks
                    for c in range(NCH):
                        base = c * SEC
                        nc.vector._custom_dve(
                            lif_op,
                            out=zt[:, base + BLK:base + SEC],
                            in0=zt[:, base:base + KB * BLK],
                            in1=zt[:, base + BLK:base + SEC],
                            s0=a_t[:, c:c + 1],
                            s1=ri_t[:, c:c + 1],
                        )

                    # state boundary into the next tile (GPSIMD: off both the
                    # DVE chain and the ACT queue, hidden under the next ops)
                    if g + 1 < G:
                        znext = zts[g + 1]
                        for c in range(NCH):
                            nc.gpsimd.tensor_copy(
                                znext[:, c * SEC:c * SEC + BLK],
                                zt[:, c * SEC + KB * BLK:(c + 1) * SEC],
                            )

                    # spikes: s = (Z > 1) as exact 0/1 u8
                    st = spool.tile([128, NCH * KB * BLK], u8)
                    nc.scalar.activation(
                        st[:].rearrange("p (c y) -> p c y", c=NCH),
                        zt[:].rearrange("p (c y) -> p c y", c=NCH)[:, :, BLK:],
                        mybir.ActivationFunctionType.Sigmoid,
                        bias=nb_t[:, 0:1],
                        scale=1.0e30,
                    )
                    nc.sync.dma_start(sg[g], st[:])

                    if g + PF + 1 < G:
                        alloc_and_fetch(g + PF + 1)
                    del zts[g]


def _get_nc(n_steps=T):
    if n_steps not in _NC_CACHE:
        _NC_CACHE[n_steps] = _build_nc(n_steps)
    return _NC_CACHE[n_steps]


def _derive_params(tau_raw, r_raw):
    """Per-neuron constants, fp32 softplus path matching jax CPU exactly."""
    tr = np.asarray(tau_raw, dtype=np.float32)
    rr = np.asarray(r_raw, dtype=np.float32)
    tau = np.logaddexp(np.float32(0.0), tr).astype(np.float32) + np.float32(TAU_MIN)
    alpha = np.exp(-np.float32(DT) / tau).astype(np.float32)
    r = np.logaddexp(np.float32(0.0), rr).astype(np.float32) + np.float32(R_MIN)
    beta = np.float32(1.0) - alpha
    bprime = beta / r
    # C1 = 1/bprime = r/beta; z0 = (1 - 1/beta)/alpha + 1 in f64 then f32
    c1 = (r.astype(np.float64) / beta.astype(np.float64)).astype(np.float32)
    z0 = (
        (1.0 - 1.0 / beta.astype(np.float64)) / alpha.astype(np.float64) + 1.0
    ).astype(np.float32)
    return alpha, c1, z0


def _core_inputs(x, alpha, c1, z0, core, n_steps):
    G = n_steps // KB
    sl = slice(core * NLOC, (core + 1) * NLOC)
    # x[:, :, sl] is [T, B, 512] -> [G, KB(t), B(b), NCH(c), 128(p)]
    #   -> device slab [G, 128(p), NCH(c), KB(t), B(b)]
    xs = x[:n_steps, :, sl].reshape(G, KB, B, NCH, 128)
    xg = np.ascontiguousarray(xs.transpose(0, 4, 3, 1, 2), dtype=np.float32)
    xg = xg.reshape(G, 128, NCH * KB * BLK)

    al = alpha[sl].reshape(NCH, 128)     # [c, p]
    a4 = np.ascontiguousarray(al.T, dtype=np.float32)          # [p, c]
    r4 = np.ascontiguousarray(c1[sl].reshape(NCH, 128).T, dtype=np.float32)
    # z0 block: [p, (c, b)] broadcast along b
    z0l = z0[sl].reshape(NCH, 128).T      # [p, c]
    z0b = np.ascontiguousarray(
        np.broadcast_to(z0l[:, :, None], (128, NCH, BLK)), dtype=np.float32
    ).reshape(128, NCH * BLK)
    return {"xg": xg, "z0": z0b, "alpha4": a4, "rinv4": r4}


def _run(x, tau_raw, r_raw, n_steps=T, trace=False, **run_kwargs):
    from concourse.bass_utils import run_bass_kernel_spmd

    alpha, c1, z0 = _derive_params(tau_raw, r_raw)
    in_maps = [
        _core_inputs(x, alpha, c1, z0, c, n_steps) for c in range(NCORES)
    ]
    nc = _get_nc(n_steps)
    res = run_bass_kernel_spmd(
        nc, in_maps, core_ids=list(range(NCORES)), trace=trace, **run_kwargs
    )
    G = n_steps // KB
    shards = []
    for c in range(NCORES):
        sgc = res.results[c]["sg"].reshape(G, 128, NCH, KB, BLK)
        # [g, p, c, t, b] -> [t_abs, b, n_local = c*128 + p]
        sc = sgc.transpose(0, 3, 4, 2, 1).reshape(n_steps, B, NLOC)
        shards.append(sc)
    out = np.concatenate(shards, axis=-1).astype(np.float32)
    return out, res


def kernel(x, tau_raw, r_raw):
    x = np.asarray(x, dtype=np.float32)
    tau_raw = np.asarray(tau_raw, dtype=np.float32)
    r_raw = np.asarray(r_raw, dtype=np.float32)
    last = None
    for attempt in range(3):
        try:
            out, _ = _run(x, tau_raw, r_raw)
            return out
        except Exception as e:  # transient NRT device errors observed rarely
            last = e
            import time as _time

            _time.sleep(2.0 * (attempt + 1))
    raise last


# revision 8
# speedup vs baseline: 1.9598x; 1.2668x over previous
"""Trainium2 Bass kernel for nn_CP_LIF (LIF neurons, softplus-parameterized
tau / soft-reset, surrogate-gradient spike forward = hard threshold).

Reference semantics per step (v-space, fp32):
    v   = alpha*v + (1-alpha)*x_t          # alpha = exp(-1/tau), per-neuron
    s   = (v - 1 > 0)                      # forward value of surrogate spike
    v   = v - s*r                          # soft reset, per-neuron r

Device math (P-space): Z := (v' - 1)/(1-alpha) + 1, so the threshold is the
constant 1 and the input current is RAW x (no per-neuron input scaling):
    Z_{t+1} = ((Z_t - 1) - (Z_t > 1)*C1) * C0 + x_{t+1}
    s_t     = (Z_t > 1)
with per-neuron constants C0 = alpha, C1 = 1/bprime (bprime = (1-alpha)/r).

The key trick: a single custom DVE instruction evaluates a whole GROUP of
timesteps of the recurrence via in-instruction self-feedback. Per neuron
chunk, the Z tile holds 1+KB blocks of 128 batch columns:
[Z_t0 | x_{t0+1} .. x_{t0+KB}]. The op's in0 AP covers blocks 0..KB-1 while
its out (and in1) AP covers blocks 1..KB — the DVE streams the free
dimension in order at 1 elem/cycle, so the output block written ~120 cycles
earlier is re-read as the next step's state (verified bit-exact on HW).
x is overwritten by Z in place.

Engines per group (n-major layout, 4 chunks of 128 neurons):
    DVE   : 4 chunk ops, FD = KB*128 each    (the entire recurrence)
    GPSIMD: 4 tiny boundary copies Z_tKB -> next tile block 0 (hidden)
    ACT   : spikes = Sigmoid(1e30*Z - 1e30) -> uint8, all 4 chunks at once
    PE    : completely idle
    DMA   : x in on the SP HWDGE ring (prefetched 2 groups ahead),
            spikes out on the ACT HWDGE ring

Group sizes ramp up (2,3,5,10,10,...) so the pipeline fill is one small DMA
instead of a full-size group. Steady state is HBM-bandwidth-bound
(~330 KB per step per core vs ~358 GB/s per-NeuronCore limit).

Sharding: neurons split 8 ways (512/core), batch full on every core; no
cross-core communication. Measured ~0 flipped spikes vs the fp32 CPU
reference on the full 100x128x4096 problem.
"""

import sys

import numpy as np

if "/opt/trn_rl_repo" not in sys.path:
    sys.path.insert(0, "/opt/trn_rl_repo")

T, B, N = 100, 128, 4096
NCORES = 8
NLOC = N // NCORES          # 512 neurons per core
NCH = NLOC // 128           # 4 partition-chunks of the neuron dim
BLK = 128                   # batch block width (one timestep column block)

DT = 1.0
V_TH = 1.0
TAU_MIN = 1e-3
R_MIN = 1e-6

KB = 10                     # steady-state timesteps per DVE instruction
RAMP = (2, 3, 5)            # pipeline-fill group sizes
PF = 2                      # x-DMA prefetch depth (groups ahead)

_NC_CACHE = {}
_LIF_OP = None


def group_sizes(n_steps):
    """Ramped group sizes summing to n_steps (fill amortization)."""
    gs = []
    rem = n_steps
    for r in RAMP:
        if rem >= r + KB:
            gs.append(r)
            rem -= r
        else:
            break
    assert rem % KB == 0 or rem <= KB
    while rem > 0:
        k = min(KB, rem)
        gs.append(k)
        rem -= k
    return gs


def _register_lif_op():
    """Custom DVE op: out = ((in0 - 1) - (in0 > 1)*C1)*C0 + in1.

    With in0 = Z_t (prev state block), in1 = x_{t+1}, C0 = alpha (per
    partition), C1 = 1/bprime (per partition) this computes Z_{t+1}; the
    in0/out APs overlap shifted by one 128-col block so one instruction
    evaluates KB serial timesteps.
    """
    global _LIF_OP
    if _LIF_OP is not None:
        return _LIF_OP
    import concourse.dve_ops as dve_ops
    from concourse.dve_ops import DveOp, OPS, CUSTOM_DVE_SPECS, _SUB_OPCODE_FOR_NAME
    from concourse.dve_spec import Spec, Src0, Src1, C0, C1, One, lower
    from concourse.dve_uop import DveOpSpec

    name = "LIF_STREAM_ANT"
    if name in _SUB_OPCODE_FOR_NAME:
        _LIF_OP = next(op for op in OPS if op.name == name)
        return _LIF_OP

    z = Src0
    s = z > One
    spec = Spec(
        body=((z - One) - s * C1) * C0 + Src1,
        reference=lambda in0, in1, s0, s1, imm2: (
            ((in0 - 1.0) - (in0 > 1.0).astype(np.float32) * s1) * s0 + in1
        ).astype(np.float32),
    )
    row = dve_ops._CUSTOM_DVE_ROW_BASE + len(OPS)
    assert row < 0x20
    shas = {}
    for ver in ("v3", "v4"):
        tmp = DveOpSpec(name=name, opcode=row, uops=lower(spec, ver=ver),
                        rd1_en=True)
        shas[ver] = tmp.sha(ver)
    op = DveOp(name, spec, subdim=False, uops_sha=shas)
    OPS.append(op)
    CUSTOM_DVE_SPECS[name] = spec
    _SUB_OPCODE_FOR_NAME[name] = row
    _LIF_OP = op
    return op


def _build_nc(n_steps=T):
    import concourse.bacc as bacc
    import concourse.tile as tile
    from concourse import mybir

    f32 = mybir.dt.float32
    u8 = mybir.dt.uint8

    lif_op = _register_lif_op()

    nc = bacc.Bacc("TRN2", target_bir_lowering=False, debug=False)

    xf = nc.dram_tensor("xf", [128, NCH * n_steps * BLK], f32,
                        kind="ExternalInput").ap()
    z0 = nc.dram_tensor("z0", [128, NCH * BLK], f32, kind="ExternalInput").ap()
    alpha4 = nc.dram_tensor("alpha4", [128, NCH], f32, kind="ExternalInput").ap()
    rinv4 = nc.dram_tensor("rinv4", [128, NCH], f32, kind="ExternalInput").ap()
    sf = nc.dram_tensor("sf", [128, NCH * n_steps * BLK], u8,
                        kind="ExternalOutput").ap()

    _emit(nc, tile, mybir, lif_op, xf, z0, alpha4, rinv4, sf, n_steps, reps=1)

    nc.compile()
    return nc


def _emit(nc, tile, mybir, lif_op, xf, z0, alpha4, rinv4, sf, n_steps, reps=1):
    f32 = mybir.dt.float32
    u8 = mybir.dt.uint8
    from contextlib import nullcontext

    gs = group_sizes(n_steps)
    G = len(gs)
    t0s = [sum(gs[:i]) for i in range(G)]   # start step of each group
    SECMAX = (KB + 1) * BLK

    xf3 = xf.rearrange("p (c t) -> p c t", c=NCH)    # t in units of BLK cols
    sf3 = sf.rearrange("p (c t) -> p c t", c=NCH)

    with tile.TileContext(nc) as tc:
        with (
            tc.tile_pool(name="const", bufs=1) as const,
            tc.tile_pool(name="zp", bufs=PF + 3) as zpool,
            tc.tile_pool(name="sp", bufs=3) as spool,
        ):
            a_t = const.tile([128, NCH], f32)
            nc.sync.dma_start(a_t[:], alpha4)
            ri_t = const.tile([128, NCH], f32)
            nc.sync.dma_start(ri_t[:], rinv4)
            nb_t = const.tile([128, 1], f32)
            nc.gpsimd.memset(nb_t[:], -1.0e30)
            guard_t = const.tile([128, NCH], f32)

            rep_cm = tc.For_i(0, reps, 1) if reps > 1 else nullcontext()
            with rep_cm:
                zts = {}

                def alloc_and_fetch(i):
                    kb = gs[i]
                    sec = (kb + 1) * BLK
                    zt = zpool.tile([128, NCH * SECMAX], f32)
                    zts[i] = zt
                    dst = zt[:].rearrange("p (c y) -> p c y", c=NCH)[
                        :, :, BLK:sec
                    ]
                    src = xf3[:, :, t0s[i] * BLK:(t0s[i] + kb) * BLK]
                    nc.sync.dma_start(dst, src)

                # prologue: prefetch x for the first PF+1 groups, seed state
                for i in range(min(PF + 1, G)):
                    alloc_and_fetch(i)
                dst = zts[0][:].rearrange("p (c y) -> p c y", c=NCH)[:, :, :BLK]
                nc.sync.dma_start(dst, z0.rearrange("p (c b) -> p c b", c=NCH))

                for g in range(G):
                    zt = zts[g]
                    kb = gs[g]
                    sec = (kb + 1) * BLK

                    # Dependency guard: the last x block of each chunk is read
                    # only via in1, which aliases the op's own out region —
                    # Tile does not derive the DMA->op edge for it. This tiny
                    # read of one column per chunk forces the wait; the real
                    # ops follow in DVE program order.
                    nc.vector.tensor_copy(
                        guard_t[:].rearrange("p (c y) -> p c y", y=1),
                        zt[:].rearrange("p (c y) -> p c y", c=NCH)[
                            :, :, kb * BLK:kb * BLK + 1
                        ],
                    )

                    # the recurrence: one DVE op per chunk, kb steps each,
                    # in-instruction self-feedback across the kb blocks
                    for c in range(NCH):
                        base = c * SECMAX
                        nc.vector._custom_dve(
                            lif_op,
                            out=zt[:, base + BLK:base + sec],
                            in0=zt[:, base:base + kb * BLK],
                            in1=zt[:, base + BLK:base + sec],
                            s0=a_t[:, c:c + 1],
                            s1=ri_t[:, c:c + 1],
                        )

                    # state boundary into the next tile (GPSIMD: off both the
                    # DVE chain and the ACT queue, hidden under the next ops)
                    if g + 1 < G:
                        znext = zts[g + 1]
                        for c in range(NCH):
                            nc.gpsimd.tensor_copy(
                                znext[:, c * SECMAX:c * SECMAX + BLK],
                                zt[:, c * SECMAX + kb * BLK:c * SECMAX + sec],
                            )

                    # spikes: s = (Z > 1) as exact 0/1 u8
                    st = spool.tile([128, NCH * KB * BLK], u8)
                    nc.scalar.activation(
                        st[:].rearrange("p (c y) -> p c y", c=NCH)[
                            :, :, :kb * BLK
                        ],
                        zt[:].rearrange("p (c y) -> p c y", c=NCH)[
                            :, :, BLK:sec
                        ],
                        mybir.ActivationFunctionType.Sigmoid,
                        bias=nb_t[:, 0:1],
                        scale=1.0e30,
                    )
                    # spikes out on the ACT HWDGE ring (overlaps the x-in ring)
                    nc.scalar.dma_start(
                        sf3[:, :, t0s[g] * BLK:(t0s[g] + kb) * BLK],
                        st[:].rearrange("p (c y) -> p c y", c=NCH)[
                            :, :, :kb * BLK
                        ],
                    )

                    if g + PF + 1 < G:
                        alloc_and_fetch(g + PF + 1)
                    del zts[g]


def _get_nc(n_steps=T):
    if n_steps not in _NC_CACHE:
        _NC_CACHE[n_steps] = _build_nc(n_steps)
    return _NC_CACHE[n_steps]


def _derive_params(tau_raw, r_raw):
    """Per-neuron constants, fp32 softplus path matching jax CPU exactly."""
    tr = np.asarray(tau_raw, dtype=np.float32)
    rr = np.asarray(r_raw, dtype=np.float32)
    tau = np.logaddexp(np.float32(0.0), tr).astype(np.float32) + np.float32(TAU_MIN)
    alpha = np.exp(-np.float32(DT) / tau).astype(np.float32)
    r = np.logaddexp(np.float32(0.0), rr).astype(np.float32) + np.float32(R_MIN)
    beta = np.float32(1.0) - alpha
    # C1 = 1/bprime = r/beta; z0 = (1 - 1/beta)/alpha + 1 in f64 then f32
    c1 = (r.astype(np.float64) / beta.astype(np.float64)).astype(np.float32)
    z0 = (
        (1.0 - 1.0 / beta.astype(np.float64)) / alpha.astype(np.float64) + 1.0
    ).astype(np.float32)
    return alpha, c1, z0


def _core_inputs(x, alpha, c1, z0, core, n_steps):
    sl = slice(core * NLOC, (core + 1) * NLOC)
    # x[:, :, sl] is [T, B, 512]; device wants [p, (c, t, b)] flat
    xs = x[:n_steps, :, sl].reshape(n_steps, B, NCH, 128)
    xfl = np.ascontiguousarray(xs.transpose(3, 2, 0, 1), dtype=np.float32)
    xfl = xfl.reshape(128, NCH * n_steps * BLK)

    a4 = np.ascontiguousarray(alpha[sl].reshape(NCH, 128).T, dtype=np.float32)
    r4 = np.ascontiguousarray(c1[sl].reshape(NCH, 128).T, dtype=np.float32)
    z0l = z0[sl].reshape(NCH, 128).T      # [p, c]
    z0b = np.ascontiguousarray(
        np.broadcast_to(z0l[:, :, None], (128, NCH, BLK)), dtype=np.float32
    ).reshape(128, NCH * BLK)
    return {"xf": xfl, "z0": z0b, "alpha4": a4, "rinv4": r4}


def _run(x, tau_raw, r_raw, n_steps=T, trace=False, **run_kwargs):
    from concourse.bass_utils import run_bass_kernel_spmd

    alpha, c1, z0 = _derive_params(tau_raw, r_raw)
    in_maps = [
        _core_inputs(x, alpha, c1, z0, c, n_steps) for c in range(NCORES)
    ]
    nc = _get_nc(n_steps)
    res = run_bass_kernel_spmd(
        nc, in_maps, core_ids=list(range(NCORES)), trace=trace, **run_kwargs
    )
    shards = []
    for c in range(NCORES):
        sfl = res.results[c]["sf"].reshape(128, NCH, n_steps, BLK)
        # [p, c, t, b] -> [t, b, n_local = c*128 + p]
        sc = sfl.transpose(2, 3, 1, 0).reshape(n_steps, B, NLOC)
        shards.append(sc)
    out = np.concatenate(shards, axis=-1).astype(np.float32)
    return out, res


def kernel(x, tau_raw, r_raw):
    x = np.asarray(x, dtype=np.float32)
    tau_raw = np.asarray(tau_raw, dtype=np.float32)
    r_raw = np.asarray(r_raw, dtype=np.float32)
    last = None
    for attempt in range(3):
        try:
            out, _ = _run(x, tau_raw, r_raw)
            return out
        except Exception as e:  # transient NRT device errors observed rarely
            last = e
            import time as _time

            _time.sleep(2.0 * (attempt + 1))
    raise last
